# revision 30
# baseline (speedup 1.0000x reference)
"""AdaptiveClusteringAttention TRN2 kernel.

Data-parallel over batch: b=8 rows -> 8 NeuronCores, one row per core,
weights replicated. No collectives.

Per-core math (n=4096 tokens, d=1024, C=256 clusters, H=16 heads, dh=64):
  xc[c,:]   = sum_{t: cluster[t]=c} x[t,:]          (onehot matmul)
  cnt[c]    = |{t: cluster[t]=c}|
  xm[c,:]   = xc[c,:] / max(cnt[c], .5)
  kc        = xm @ w_k ; vc = xm @ w_v              (segmean commutes with proj)
  qh        = x @ w_q
  s[t,c]    = qh_h[t] . kc_h[c] / 8
  attn      = softmax(s + log cnt)                  (count-weighted softmax)
  out       = attn @ vc ; y = out @ w_proj + b_proj

Layouts: everything d-major ("transposed") so x^T is the only transpose,
done via f32->bf16 cast-DMA + XBAR DMA-transpose. Matmuls are bf16 on the
q/score/attn path and float32r elsewhere. exp is fused into the scores
PSUM eviction (scale=1/8, bias=log-counts per partition). sum-exp comes
free from a ones column appended to vc; 1/sumexp is broadcast across
partitions with a K=1 matmul.
"""

import os
import sys

import numpy as np

for _p in ("/opt/trn_rl_repo", os.path.expanduser("~/.axon_site/_ro/trn_rl_repo")):
    if os.path.isdir(_p) and _p not in sys.path:
        sys.path.append(_p)

import concourse.bass as bass  # noqa: E402
import concourse.mybir as mybir  # noqa: E402
import concourse.tile as tile  # noqa: E402
from concourse import bacc  # noqa: E402
from concourse.masks import make_identity  # noqa: E402

FP32 = mybir.dt.float32
BF16 = mybir.dt.bfloat16
I32 = mybir.dt.int32

N, D, C, H, DH, P = 4096, 1024, 256, 16, 64, 128
NJ = N // P          # 32 token row-tiles
NK = D // P          # 8 contraction chunks
TCH = 512            # token chunk for the attention phase
NCH = N // TCH       # 8 chunks
NMT = TCH // P       # 4 token subtiles per chunk

TRACE = False
LAST_RESULTS = None


def build_nc():
    nc = bacc.Bacc("TRN2", target_bir_lowering=False, debug=False)

    x_d = nc.dram_tensor("x", [N, D], FP32, kind="ExternalInput").ap()
    cl_d = nc.dram_tensor("cluster", [N], I32, kind="ExternalInput").ap()
    wq_d = nc.dram_tensor("w_q", [D, D], FP32, kind="ExternalInput").ap()
    wk_d = nc.dram_tensor("w_k", [D, D], FP32, kind="ExternalInput").ap()
    wv_d = nc.dram_tensor("w_v", [D, D], FP32, kind="ExternalInput").ap()
    wp_d = nc.dram_tensor("w_proj", [D, D], FP32, kind="ExternalInput").ap()
    bp_d = nc.dram_tensor("b_proj", [1, D], FP32, kind="ExternalInput").ap()
    out_d = nc.dram_tensor("out", [N, D], FP32, kind="ExternalOutput").ap()

    with tile.TileContext(nc) as tc:
        with (
            tc.tile_pool(name="dram", bufs=1, space="DRAM") as dram,
            tc.tile_pool(name="wts", bufs=1) as wts,
        ):
            xbf_d = dram.tile([N, D], BF16)

            # ---- constants ----
            iota_i = wts.tile([P, C], I32, tag="iota_i")
            nc.gpsimd.iota(iota_i[:], pattern=[[1, C]], base=0, channel_multiplier=0)
            iota_b = wts.tile([P, C], BF16, tag="iota_b")
            nc.vector.tensor_copy(iota_b[:], iota_i[:])
            ident = wts.tile([32, 32], BF16, tag="ident")
            make_identity(nc, ident[:])
            ones_col = wts.tile([P, 1], BF16, tag="ones_col")
            nc.vector.memset(ones_col[:], 1.0)
            ones_row = wts.tile([1, 64], BF16, tag="ones_row")
            nc.vector.memset(ones_row[:], 1.0)

            bp_sb = wts.tile([1, D], FP32, tag="bp_sb")
            nc.sync.dma_start(out=bp_sb[:], in_=bp_d)
            b_bc = wts.tile([P, D], FP32, tag="b_bc")
            nc.gpsimd.partition_broadcast(b_bc[:], bp_sb[:])

            cl_i = wts.tile([NJ, P], I32, tag="cl_i")
            nc.sync.dma_start(out=cl_i[:], in_=cl_d.rearrange("(a b) -> a b", b=P))
            cl_b = wts.tile([NJ, P], BF16, tag="cl_b")
            nc.vector.tensor_copy(cl_b[:], cl_i[:])
            clusT = wts.tile([P, NJ], FP32, tag="clusT")
            with tc.tile_pool(name="psct", bufs=1, space="PSUM") as psct:
                ct_ps = psct.tile([P, NJ], BF16, tag="ct")
                nc.tensor.transpose(ct_ps[:], cl_b[:], ident[:])
                nc.vector.tensor_copy(clusT[:], ct_ps[:])

            # weights (bf16 via cast-DMA)
            wq_sb, wp_sb = [], []
            for k in range(NK):
                t = wts.tile([P, D], BF16, tag=f"wq{k}", name=f"wq{k}")
                nc.gpsimd.dma_start(out=t[:], in_=wq_d[k * P:(k + 1) * P, :])
                wq_sb.append(t)
                t = wts.tile([P, D], BF16, tag=f"wp{k}", name=f"wp{k}")
                nc.gpsimd.dma_start(out=t[:], in_=wp_d[k * P:(k + 1) * P, :])
                wp_sb.append(t)

            # ---- phase A: stream x (f32 DMA + DVE cast), onehot + counts ----
            xcm = [wts.tile([P, C], BF16, tag=f"xcm{m}", name=f"xcm{m}")
                   for m in range(NK)]
            cnt_sb = wts.tile([1, C], FP32, tag="cnt_sb")
            logc = wts.tile([P, 2], FP32, tag="logc")
            with (
                tc.tile_pool(name="psA", bufs=1, space="PSUM") as psA,
                tc.tile_pool(name="psAm", bufs=2, space="PSUM") as psAm,
                tc.tile_pool(name="xin", bufs=1) as xin,
                tc.tile_pool(name="xf32", bufs=6) as xf32,
                tc.tile_pool(name="ohp", bufs=1) as ohp,
            ):
                pcnt = psA.tile([1, C], FP32, tag="cnt")
                pcT = [psA.tile([P, 1], FP32, tag=f"cntT{i}", name=f"pcT{i}")
                       for i in range(2)]
                xall, ohall = [], []
                for j in range(NJ):
                    xf = xf32.tile([P, D], FP32, tag="xf")
                    nc.sync.dma_start(out=xf[:], in_=x_d[j * P:(j + 1) * P, :])
                    xj = xin.tile([P, D], BF16, tag=f"xj{j}", name=f"xj{j}")
                    nc.vector.tensor_copy(xj[:], xf[:])
                    nc.scalar.dma_start(out=xbf_d[j * P:(j + 1) * P, :], in_=xj[:])
                    oh = ohp.tile([P, C], BF16, tag=f"oh{j}", name=f"oh{j}")
                    nc.vector.tensor_scalar(
                        oh[:], iota_b[:], clusT[:, j:j + 1], None,
                        mybir.AluOpType.is_equal,
                    )
                    st, sp = (j == 0), (j == NJ - 1)
                    nc.tensor.matmul(pcnt[:], ones_col[:], oh[:], start=st, stop=sp)
                    for mc in range(2):
                        nc.tensor.matmul(
                            pcT[mc][:], oh[:, mc * P:(mc + 1) * P],
                            ones_col[:], start=st, stop=sp,
                        )
                    xall.append(xj)
                    ohall.append(oh)

                # counts -> inv (row + bcast), log-counts (column layout)
                nc.scalar.copy(cnt_sb[:], pcnt[:])
                cm_row = wts.tile([1, C], FP32, tag="cm_row")
                nc.vector.tensor_scalar_max(cm_row[:], cnt_sb[:], 0.5)
                inv_row = wts.tile([1, C], FP32, tag="inv_row")
                nc.vector.reciprocal(inv_row[:], cm_row[:])
                inv_bc = wts.tile([P, C], FP32, tag="inv_bc")
                nc.gpsimd.partition_broadcast(inv_bc[:], inv_row[:])

                cnt_col = wts.tile([P, 2], FP32, tag="cnt_col")
                for mc in range(2):
                    nc.scalar.copy(cnt_col[:, mc:mc + 1], pcT[mc][:])
                cm_col = wts.tile([P, 2], FP32, tag="cm_col")
                nc.vector.tensor_scalar_max(cm_col[:], cnt_col[:], 0.5)
                lg_col = wts.tile([P, 2], FP32, tag="lg_col")
                nc.scalar.activation(lg_col[:], cm_col[:],
                                     mybir.ActivationFunctionType.Ln)
                msk = wts.tile([P, 2], FP32, tag="msk")
                nc.vector.tensor_scalar(
                    msk[:], cnt_col[:], 0.5, 30.0,
                    mybir.AluOpType.is_lt, mybir.AluOpType.mult,
                )
                nc.vector.tensor_sub(logc[:], lg_col[:], msk[:])

                # xm^T = xc^T * inv  (d-major cluster means)
                for m in range(NK):
                    pxc = psAm.tile([P, C], FP32, tag="pxc")
                    for j in range(NJ):
                        nc.tensor.matmul(
                            pxc[:], xall[j][:, m * P:(m + 1) * P], ohall[j][:],
                            start=(j == 0), stop=(j == NJ - 1),
                        )
                    nc.vector.tensor_mul(xcm[m][:], pxc[:], inv_bc[:])

            # ---- phase B: kc^T and vc (with ones column) ----
            kc_sb = [wts.tile([P, C], BF16, tag=f"kc{m}", name=f"kc{m}")
                     for m in range(NK)]
            vca = [wts.tile([P, 16 * 65], BF16, tag=f"vca{i}", name=f"vca{i}")
                   for i in range(2)]
            for i in range(2):
                va = vca[i].rearrange("p (h e) -> p h e", e=65)
                nc.vector.memset(va[:, :, 64:65], 1.0)
            with (
                tc.tile_pool(name="psBk", bufs=2, space="PSUM") as psBk,
                tc.tile_pool(name="psBv", bufs=4, space="PSUM") as psBv,
                tc.tile_pool(name="wkv", bufs=1) as wkv,
            ):
                wk_sb, wv_sb = [], []
                for k in range(NK):
                    t = wkv.tile([P, D], BF16, tag=f"wk{k}", name=f"wk{k}")
                    nc.gpsimd.dma_start(out=t[:], in_=wk_d[k * P:(k + 1) * P, :])
                    wk_sb.append(t)
                    t = wkv.tile([P, D], BF16, tag=f"wv{k}", name=f"wv{k}")
                    nc.gpsimd.dma_start(out=t[:], in_=wv_d[k * P:(k + 1) * P, :])
                    wv_sb.append(t)
                for m in range(NK):
                    pk = psBk.tile([P, C], FP32, tag="pk")
                    for k in range(NK):
                        nc.tensor.matmul(
                            pk[:], wk_sb[k][:, m * P:(m + 1) * P],
                            xcm[k][:], start=(k == 0), stop=(k == NK - 1),
                        )
                    nc.vector.tensor_copy(kc_sb[m][:], pk[:])
                for mc in range(2):
                    va = vca[mc].rearrange("p (h e) -> p h e", e=65)
                    for nn in range(2):
                        pv = psBv.tile([P, 512], FP32, tag="pv")
                        for k in range(NK):
                            nc.tensor.matmul(
                                pv[:], xcm[k][:, mc * P:(mc + 1) * P],
                                wv_sb[k][:, nn * 512:(nn + 1) * 512],
                                start=(k == 0), stop=(k == NK - 1),
                            )
                        nc.vector.tensor_copy(
                            va[:, nn * 8:(nn + 1) * 8, 0:64],
                            pv.rearrange("p (h e) -> p h e", e=64),
                        )

            # ---- phase C/D: per token-chunk attention + output proj ----
            with (
                tc.tile_pool(name="xtp", bufs=2) as xtp,
                tc.tile_pool(name="qhp", bufs=2) as qhp,
                tc.tile_pool(name="expp", bufs=4) as expp,
                tc.tile_pool(name="sep", bufs=2) as sep,
                tc.tile_pool(name="otp", bufs=2) as otp,
                tc.tile_pool(name="finp", bufs=4) as finp,
                tc.tile_pool(name="psq", bufs=1, space="PSUM") as psq,
                tc.tile_pool(name="pss", bufs=1, space="PSUM") as pss,
                tc.tile_pool(name="psav", bufs=2, space="PSUM") as psav,
                tc.tile_pool(name="psf", bufs=2, space="PSUM") as psf,
            ):
                for ch in range(NCH):
                    t0 = ch * TCH
                    xT = []
                    for k in range(NK):
                        t = xtp.tile([P, TCH], BF16, tag=f"xt{k}", name=f"xt{k}")
                        nc.sync.dma_start_transpose(
                            out=t[:], in_=xbf_d[t0:t0 + TCH, k * P:(k + 1) * P]
                        )
                        xT.append(t)
                    qh = []
                    for m in range(NK):
                        pq = psq.tile([P, TCH], FP32, tag="pq")
                        for k in range(NK):
                            nc.tensor.matmul(
                                pq[:], wq_sb[k][:, m * P:(m + 1) * P], xT[k][:],
                                start=(k == 0), stop=(k == NK - 1),
                            )
                        qt = qhp.tile([P, TCH], BF16, tag=f"qh{m}", name=f"qh{m}")
                        nc.vector.tensor_copy(qt[:], pq[:])
                        qh.append(qt)

                    outT = [otp.tile([P, TCH], BF16, tag=f"ot{m}", name=f"ot{m}")
                            for m in range(NK)]
                    se_eo = [sep.tile([1, (H // 2) * TCH], BF16, tag=f"se{i}",
                                      name=f"se{i}") for i in range(2)]
                    for h in range(H):
                        m, off = h // 2, (h % 2) * 64
                        ex = []
                        for mc in range(2):
                            ps = pss.tile([P, TCH], FP32, tag=f"s{mc}")
                            nc.tensor.matmul(
                                ps[:],
                                kc_sb[m][off:off + 64, mc * P:(mc + 1) * P],
                                qh[m][off:off + 64, :],
                                start=True, stop=True,
                            )
                            e = expp.tile([P, TCH], BF16, tag="exp")
                            nc.scalar.activation(
                                e[:], ps[:], mybir.ActivationFunctionType.Exp,
                                bias=logc[:, mc:mc + 1], scale=0.125,
                            )
                            ex.append(e)
                        pav = psav.tile([65, TCH], FP32, tag="av")
                        for mc in range(2):
                            nc.tensor.matmul(
                                pav[:], vca[mc][:, h * 65:(h + 1) * 65], ex[mc][:],
                                start=(mc == 0), stop=(mc == 1),
                            )
                        se_dst = se_eo[h % 2][0:1, (h // 2) * TCH:
                                             (h // 2 + 1) * TCH]
                        if h % 2 == 0:
                            nc.vector.tensor_copy(se_dst, pav[64:65, :])
                            nc.scalar.copy(outT[m][off:off + 64, :], pav[0:64, :])
                        else:
                            nc.scalar.copy(se_dst, pav[64:65, :])
                            nc.vector.tensor_copy(
                                outT[m][off:off + 64, :], pav[0:64, :]
                            )

                    # batched 1/sumexp across all DVE lanes
                    sq = sep.tile([P, TCH // 8], FP32, tag="sq")
                    for i in range(2):
                        nc.gpsimd.dma_start(
                            out=sq[i * 64:(i + 1) * 64, :],
                            in_=se_eo[i].rearrange("a (p t) -> a p t", t=TCH),
                        )
                    rq = sep.tile([P, TCH // 8], FP32, tag="rq")
                    nc.vector.reciprocal(rq[:], sq[:])
                    rec_eo = [sep.tile([1, (H // 2) * TCH], BF16, tag=f"rec{i}",
                                       name=f"rec{i}") for i in range(2)]
                    for i in range(2):
                        nc.gpsimd.dma_start(
                            out=rec_eo[i].rearrange("a (p t) -> a p t", t=TCH),
                            in_=rq[i * 64:(i + 1) * 64, :],
                        )
                    for p in range(H // 2):
                        pbc = psf.tile([P, TCH], FP32, tag="pf")
                        for i in range(2):
                            nc.tensor.matmul(
                                pbc[i * 64:(i + 1) * 64, :], ones_row[:],
                                rec_eo[i][0:1, p * TCH:(p + 1) * TCH],
                                start=True, stop=True,
                            )
                        nc.vector.tensor_mul(outT[p][:], outT[p][:], pbc[:])

                    for mt in range(NMT):
                        pf = [psf.tile([P, 512], FP32, tag="pf", name=f"pf{nn}")
                              for nn in range(2)]
                        for k in range(NK):
                            for nn in range(2):
                                nc.tensor.matmul(
                                    pf[nn][:],
                                    outT[k][:, mt * P:(mt + 1) * P],
                                    wp_sb[k][:, nn * 512:(nn + 1) * 512],
                                    start=(k == 0), stop=(k == NK - 1),
                                )
                        for nn in range(2):
                            fin = finp.tile([P, 512], FP32, tag="fin")
                            nc.vector.tensor_add(
                                fin[:], pf[nn][:],
                                b_bc[:, nn * 512:(nn + 1) * 512]
                            )
                            nc.gpsimd.dma_start(
                                out=out_d[t0 + mt * P:t0 + (mt + 1) * P,
                                          nn * 512:(nn + 1) * 512],
                                in_=fin[:],
                            )
    nc.compile()
    return nc


_NC = None


def _get_nc():
    global _NC
    if _NC is None:
        _NC = build_nc()
    return _NC


def make_in_maps(cluster, q, w_q, w_kv, w_proj, b_proj):
    cluster = np.ascontiguousarray(np.asarray(cluster).astype(np.int32, copy=False))
    q = np.asarray(q, dtype=np.float32)
    w_q = np.ascontiguousarray(np.asarray(w_q, dtype=np.float32))
    w_kv = np.asarray(w_kv, dtype=np.float32)
    w_k = np.ascontiguousarray(w_kv[:, :D])
    w_v = np.ascontiguousarray(w_kv[:, D:])
    w_proj = np.ascontiguousarray(np.asarray(w_proj, dtype=np.float32))
    b_proj = np.ascontiguousarray(
        np.asarray(b_proj, dtype=np.float32).reshape(1, D)
    )
    return [
        {
            "x": np.ascontiguousarray(q[i]),
            "cluster": cluster[i],
            "w_q": w_q,
            "w_k": w_k,
            "w_v": w_v,
            "w_proj": w_proj,
            "b_proj": b_proj,
        }
        for i in range(q.shape[0])
    ]


def kernel(cluster, q, w_q, w_kv, w_proj, b_proj):
    global LAST_RESULTS
    from concourse.bass_utils import run_bass_kernel_spmd

    nc = _get_nc()
    in_maps = make_in_maps(cluster, q, w_q, w_kv, w_proj, b_proj)
    ncores = len(in_maps)
    res = run_bass_kernel_spmd(
        nc, in_maps, core_ids=list(range(ncores)), trace=TRACE
    )
    LAST_RESULTS = res
    return np.stack([res.results[i]["out"] for i in range(ncores)], axis=0)
